# revision 4
# baseline (speedup 1.0000x reference)
"""Trainium2 Bass kernel for DenseRoutingMaskLayer (MoE routing chunk-gather), v2.

reference: route = argmax(routing_inputs, -1); out[b] = inputs[b].reshape(8, 512)[route[b]]

Pure data parallel across 8 NeuronCores (2048 rows each). v2 changes vs v1:

  * bf16 payload: the host casts inputs to bf16 (rel err ~2^-9, far under the
    2e-2 gate); the gather moves 1KB/row instead of 2KB, stores 1KB/row.
  * one-shot index pipeline: argmax + flat-index for all 2048 rows in a single
    full-width DVE pass (reduce_max / is_equal / mult / reduce_add), one PE
    transpose against eye(128), one copy+stream_shuffle -> wrapped idx tile in
    partitions 0..31.
  * 4 SWDGE queues: the wrapped idx pattern is replicated into partition
    quadrants 1..3 with three SBUF->SBUF HWDGE copies; the 8 sub-gathers run
    on queue k%4, so descriptor generation runs on all four Q7 core pairs
    concurrently instead of serializing on pair 0.

The gathered row i lands at SBUF partition i%128, col i//128; stores use a
matching strided DRAM view (host transposes back and upcasts to f32).
"""

import sys

import numpy as np

try:
    import concourse  # noqa: F401
except ImportError:  # pragma: no cover
    sys.path.insert(0, "/opt/trn_rl_repo")

import ml_dtypes

N_CORES = 8
B_FULL = 16384
D = 4096
ROUTES = 8
RW = D // ROUTES
B_SH = B_FULL // N_CORES  # 2048
NC_COLS = B_SH // 16  # 128
NJ = B_SH // 128  # 16
NG = 8  # sub-gathers (256 rows each)

_prog_cache = {}


def _build_program():
    import concourse.bacc as bacc
    import concourse.mybir as mybir
    from concourse.library_config import mlp
    from contextlib import ExitStack

    f32 = mybir.dt.float32
    bf16 = mybir.dt.bfloat16
    i32 = mybir.dt.int32
    i16 = mybir.dt.int16
    Alu = mybir.AluOpType
    Ax = mybir.AxisListType

    rows_g = B_SH // NG  # 256
    cg = NC_COLS // NG  # 16 wrapped cols per sub-gather
    jg = NJ // NG  # 2 output j-cols per sub-gather

    nc = bacc.Bacc(
        "TRN2",
        target_bir_lowering=False,
        debug=False,
        num_devices=N_CORES,
        num_swdge_queues=4,
    )
    x = nc.dram_tensor("x", [B_SH, D], bf16, kind="ExternalInput")
    rt = nc.dram_tensor("rt", [B_SH, ROUTES], f32, kind="ExternalInput")
    wt = nc.dram_tensor("wt", [128, 16, ROUTES], f32, kind="ExternalInput")
    it = nc.dram_tensor("it", [128, 128], f32, kind="ExternalInput")
    # partition-major output: y[p, j, :] holds row j*128+p; the host
    # transposes back. Keeps every store descriptor 2KB-contiguous.
    y = nc.dram_tensor("y", [128, NJ, RW], bf16, kind="ExternalOutput")

    x_rows = x.ap().rearrange("b (r w) -> (b r) w", r=ROUTES)
    rt_n = rt.ap().rearrange("(p u) r -> p u r", u=16)  # [128, 16, 8]
    y_pjw = y.ap()

    with (
        ExitStack() as ctx,
        nc.sbuf_tensor("wt_t", [128, 16, ROUTES], f32) as wt_t,
        nc.sbuf_tensor("it_t", [128, 128], f32) as it_t,
        nc.sbuf_tensor("r_t", [128, 16, ROUTES], f32) as r_t,
        nc.sbuf_tensor("m_t", [128, 16], f32) as m_t,
        nc.sbuf_tensor("eq_t", [128, 16, ROUTES], f32) as eq_t,
        nc.sbuf_tensor("idf", [128, 16], f32) as idf,
        nc.sbuf_tensor("idx32", [32, NC_COLS], i32) as idx32,
        nc.sbuf_tensor("idx16", [128, NC_COLS], i16) as idx16,
        nc.sbuf_tensor("g_t", [128, NJ, RW], bf16) as g_t,
        nc.psum_tensor("t1", [16, NC_COLS], f32) as t1,
        nc.Block(no_gpsimd_drain=True) as block,
    ):
        s_rt = ctx.enter_context(nc.semaphore("s_rt"))
        s_wt = ctx.enter_context(nc.semaphore("s_wt"))
        s_id = ctx.enter_context(nc.semaphore("s_id"))
        s_v = ctx.enter_context(nc.semaphore("s_v"))
        s_mm = ctx.enter_context(nc.semaphore("s_mm"))
        s_q = [ctx.enter_context(nc.semaphore(f"s_q{q}")) for q in range(1, 4)]
        s_g = [ctx.enter_context(nc.semaphore(f"s_g{k}")) for k in range(NG + 1)]
        s_y = ctx.enter_context(nc.semaphore("s_y"))

        # DVE step ids: 1 big-memset (delays input loads past the library DMA),
        # 2 memset32, 3 memset16, 4 max, 5 eq, 6 mult, 7 sum, 8 copy, 9 shuffle
        S_IDF = 7
        S_IDX = 9

        @block.scalar
        def _(act):
            # hold input loads off the DMA rings while the MPC library image
            # transfers (first-gather gate); idx-ready has ~3us of slack
            act.wait_ge(s_v, 1)
            act.dma_start(r_t[:], rt_n[:]).then_inc(s_rt, 16)
            # replicate wrapped idx pattern into partition quadrant 1
            act.wait_ge(s_v, S_IDX)
            act.dma_start(idx16[32:64, :], idx16[0:32, :]).then_inc(s_q[0], 16)
            for k in (1, 3, 5):
                js = slice(k * jg, (k + 1) * jg)
                act.wait_ge(s_g[k], 16)
                act.dma_start(y_pjw[:, js, :], g_t[:, js, :]).then_inc(s_y, 16)
            act.wait_ge(s_g[8], 16)
            act.dma_start(y_pjw[:, 15:16, :], g_t[:, 15:16, :]).then_inc(s_y, 16)

        @block.vector
        def _(dve):
            k = 0

            def step(inst):
                nonlocal k
                k += 1
                inst.then_inc(s_v, 1)
                dve.wait_ge(s_v, k)

            step(dve.memset(g_t[:, 0:6, :], 0))
            step(dve.memset(idx32[:], 0))
            step(dve.memset(idx16[:], 0))
            dve.wait_ge(s_rt, 16)
            step(dve.tensor_reduce(m_t[:], r_t[:], Ax.X, Alu.max))
            step(
                dve.tensor_tensor(
                    eq_t[:],
                    r_t[:],
                    m_t[:].unsqueeze(2).broadcast_to([128, 16, ROUTES]),
                    Alu.is_equal,
                )
            )
            dve.wait_ge(s_wt, 16)
            step(dve.tensor_tensor(eq_t[:], eq_t[:], wt_t[:], Alu.mult))
            step(dve.tensor_reduce(idf[:], eq_t[:], Ax.X, Alu.add))
            dve.wait_ge(s_mm, 1)
            step(dve.tensor_copy(idx32[0:16, :], t1[:]))
            step(
                dve.stream_shuffle(
                    idx16[0:32, :],
                    idx32[0:32, :]
                    .bitcast(i16)
                    .rearrange("q (c two) -> q c two", two=2)[:, :, 0],
                    list(range(16)) * 2,
                )
            )

        @block.tensor
        def _(pe):
            pe.wait_ge(s_id, 16)
            pe.wait_ge(s_v, S_IDF)
            pe.transpose(t1[:], idf[:], it_t[:]).then_inc(s_mm, 1)

        @block.gpsimd
        def _(pool):
            pool.load_library(mlp)
            pool.wait_ge(s_v, S_IDX)
            for k in range(NG - 1):
                q = k % 4
                if 1 <= q <= 3 and k < 4:
                    pool.wait_ge(s_q[q - 1], 16)
                cs = slice(k * cg, (k + 1) * cg)
                js = slice(k * jg, (k + 1) * jg)
                pool.dma_gather(
                    g_t[:, js, :], x_rows, idx16[:, cs], rows_g, rows_g, RW,
                    single_packet=True,
                    queue_num=q,
                ).then_inc(s_g[k], 16)
            # split the final chunk so the tail store is half-size and releases earlier
            for h in range(2):
                cs = slice(112 + 8 * h, 120 + 8 * h)
                js = slice(14 + h, 15 + h)
                pool.dma_gather(
                    g_t[:, js, :], x_rows, idx16[:, cs], 128, 128, RW,
                    single_packet=True,
                    queue_num=3,
                ).then_inc(s_g[7 + h], 16)

        @block.sync
        def _(sp):
            sp.wait_ge(s_v, 1)
            sp.dma_start(wt_t[:], wt.ap()).then_inc(s_wt, 16)
            sp.dma_start(it_t[:], it.ap()).then_inc(s_id, 16)
            # replicate wrapped idx pattern into partition quadrants 2 and 3
            sp.wait_ge(s_v, S_IDX)
            sp.dma_start(idx16[64:96, :], idx16[0:32, :]).then_inc(s_q[1], 16)
            sp.dma_start(idx16[96:128, :], idx16[0:32, :]).then_inc(s_q[2], 16)
            for k in (0, 2, 4, 6):
                js = slice(k * jg, (k + 1) * jg)
                sp.wait_ge(s_g[k], 16)
                sp.dma_start(y_pjw[:, js, :], g_t[:, js, :]).then_inc(s_y, 16)
            sp.wait_ge(s_g[7], 16)
            sp.dma_start(y_pjw[:, 14:15, :], g_t[:, 14:15, :]).then_inc(s_y, 16)
            sp.wait_ge(s_y, 144)

    nc.compile()
    return nc


def _get_program():
    if "v2" not in _prog_cache:
        _prog_cache["v2"] = _build_program()
    return _prog_cache["v2"]


def _weights():
    p = np.arange(128, dtype=np.float32)[:, None, None]
    u = np.arange(16, dtype=np.float32)[None, :, None]
    r = np.arange(ROUTES, dtype=np.float32)[None, None, :]
    return np.ascontiguousarray(r + 8.0 * (p * 16.0 + u), dtype=np.float32)


def _identity():
    return np.eye(128, dtype=np.float32)


def _in_maps(inputs, routing_inputs):
    xb = inputs.astype(ml_dtypes.bfloat16)
    rt = np.ascontiguousarray(routing_inputs, dtype=np.float32)
    wt = _weights()
    it = _identity()
    return [
        {
            "x": np.ascontiguousarray(xb[c * B_SH : (c + 1) * B_SH]),
            "rt": np.ascontiguousarray(rt[c * B_SH : (c + 1) * B_SH]),
            "wt": wt,
            "it": it,
        }
        for c in range(N_CORES)
    ]


def _collect(res):
    return np.concatenate(
        [
            res.results[c]["y"].transpose(1, 0, 2).reshape(B_SH, RW)
            for c in range(N_CORES)
        ],
        axis=0,
    ).astype(np.float32)


def kernel(inputs: np.ndarray, routing_inputs: np.ndarray) -> np.ndarray:
    from concourse.bass_utils import run_bass_kernel_spmd

    nc = _get_program()
    in_maps = _in_maps(np.asarray(inputs), np.asarray(routing_inputs))
    res = None
    for attempt in range(3):
        try:
            res = run_bass_kernel_spmd(nc, in_maps, core_ids=list(range(N_CORES)))
            break
        except Exception:  # transient NRT_EXEC_UNIT_UNRECOVERABLE flakes
            if attempt == 2:
                raise
            import time

            time.sleep(2.0)
    return _collect(res)


# revision 5
# speedup vs baseline: 1.1832x; 1.1832x over previous
"""Trainium2 Bass kernel for DenseRoutingMaskLayer (MoE routing chunk-gather), v2.

reference: route = argmax(routing_inputs, -1); out[b] = inputs[b].reshape(8, 512)[route[b]]

Pure data parallel across 8 NeuronCores (2048 rows each). v2 changes vs v1:

  * bf16 payload: the host casts inputs to bf16 (rel err ~2^-9, far under the
    2e-2 gate); the gather moves 1KB/row instead of 2KB, stores 1KB/row.
  * one-shot index pipeline: argmax + flat-index for all 2048 rows in a single
    full-width DVE pass (reduce_max / is_equal / mult / reduce_add), one PE
    transpose against eye(128), one copy+stream_shuffle -> wrapped idx tile in
    partitions 0..31.
  * 4 SWDGE queues: the wrapped idx pattern is replicated into partition
    quadrants 1..3 with three SBUF->SBUF HWDGE copies; the 8 sub-gathers run
    on queue k%4, so descriptor generation runs on all four Q7 core pairs
    concurrently instead of serializing on pair 0.

The gathered row i lands at SBUF partition i%128, col i//128; stores use a
matching strided DRAM view (host transposes back and upcasts to f32).
"""

import sys

import numpy as np

try:
    import concourse  # noqa: F401
except ImportError:  # pragma: no cover
    sys.path.insert(0, "/opt/trn_rl_repo")

import ml_dtypes

N_CORES = 8
B_FULL = 16384
D = 4096
ROUTES = 8
RW = D // ROUTES
B_SH = B_FULL // N_CORES  # 2048
NC_COLS = B_SH // 16  # 128
NJ = B_SH // 128  # 16
NG = 8  # sub-gathers (256 rows each)

_prog_cache = {}


def _build_program():
    import concourse.bacc as bacc
    import concourse.mybir as mybir
    from concourse.library_config import mlp
    from contextlib import ExitStack

    f32 = mybir.dt.float32
    bf16 = mybir.dt.bfloat16
    i32 = mybir.dt.int32
    i16 = mybir.dt.int16
    Alu = mybir.AluOpType
    Ax = mybir.AxisListType

    rows_g = B_SH // NG  # 256
    cg = NC_COLS // NG  # 16 wrapped cols per sub-gather
    jg = NJ // NG  # 2 output j-cols per sub-gather

    nc = bacc.Bacc(
        "TRN2",
        target_bir_lowering=False,
        debug=False,
        num_devices=N_CORES,
        num_swdge_queues=4,
    )
    x = nc.dram_tensor("x", [B_SH, D], bf16, kind="ExternalInput")
    rt = nc.dram_tensor("rt", [B_SH, ROUTES], f32, kind="ExternalInput")
    wt = nc.dram_tensor("wt", [128, 16, ROUTES], f32, kind="ExternalInput")
    it = nc.dram_tensor("it", [128, 128], f32, kind="ExternalInput")
    # partition-major output: y[p, j, :] holds row j*128+p; the host
    # transposes back. Keeps every store descriptor 2KB-contiguous.
    y = nc.dram_tensor("y", [128, NJ, RW], bf16, kind="ExternalOutput")

    x_rows = x.ap().rearrange("b (r w) -> (b r) w", r=ROUTES)
    rt_n = rt.ap().rearrange("(p u) r -> p u r", u=16)  # [128, 16, 8]
    y_pjw = y.ap()

    with (
        ExitStack() as ctx,
        nc.sbuf_tensor("wt_t", [128, 16, ROUTES], f32) as wt_t,
        nc.sbuf_tensor("it_t", [128, 128], f32) as it_t,
        nc.sbuf_tensor("r_t", [128, 16, ROUTES], f32) as r_t,
        nc.sbuf_tensor("m_t", [128, 16], f32) as m_t,
        nc.sbuf_tensor("eq_t", [128, 16, ROUTES], f32) as eq_t,
        nc.sbuf_tensor("idf", [128, 16], f32) as idf,
        nc.sbuf_tensor("idf8", [128, 128], f32) as idf8,
        nc.sbuf_tensor("idx32", [128, NC_COLS], i32) as idx32,
        nc.sbuf_tensor("idx16", [128, NC_COLS], i16) as idx16,
        nc.sbuf_tensor("g_t", [128, NJ, RW], bf16) as g_t,
        nc.psum_tensor("t8", [128, NC_COLS], f32) as t8,
        nc.Block(no_gpsimd_drain=True) as block,
    ):
        s_rt = ctx.enter_context(nc.semaphore("s_rt"))
        s_wt = ctx.enter_context(nc.semaphore("s_wt"))
        s_id = ctx.enter_context(nc.semaphore("s_id"))
        s_v = ctx.enter_context(nc.semaphore("s_v"))
        s_mm = ctx.enter_context(nc.semaphore("s_mm"))
        s_g = [ctx.enter_context(nc.semaphore(f"s_g{k}")) for k in range(NG + 1)]
        s_y = ctx.enter_context(nc.semaphore("s_y"))

        # DVE step ids: 1 big-memset (delays input loads past the library DMA),
        # 2 max, 3 eq, 4 mult, 5 sum, 6 bcast, 7 cast32, 8 pack16
        S_IDF = 6
        S_IDX = 8

        @block.scalar
        def _(act):
            # hold input loads off the DMA rings while the MPC library image
            # transfers (first-gather gate); idx-ready has ~3us of slack
            act.wait_ge(s_v, 1)
            act.dma_start(r_t[:], rt_n[:]).then_inc(s_rt, 16)
            for k in (1, 3, 5):
                js = slice(k * jg, (k + 1) * jg)
                act.wait_ge(s_g[k], 16)
                act.dma_start(y_pjw[:, js, :], g_t[:, js, :]).then_inc(s_y, 16)
            act.wait_ge(s_g[8], 16)
            act.dma_start(y_pjw[:, 15:16, :], g_t[:, 15:16, :]).then_inc(s_y, 16)

        @block.vector
        def _(dve):
            k = 0

            def step(inst):
                nonlocal k
                k += 1
                inst.then_inc(s_v, 1)
                dve.wait_ge(s_v, k)

            step(dve.memset(g_t[:, 0:4, :], 0))
            dve.wait_ge(s_rt, 16)
            step(dve.tensor_reduce(m_t[:], r_t[:], Ax.X, Alu.max))
            step(
                dve.tensor_tensor(
                    eq_t[:],
                    r_t[:],
                    m_t[:].unsqueeze(2).broadcast_to([128, 16, ROUTES]),
                    Alu.is_equal,
                )
            )
            dve.wait_ge(s_wt, 16)
            step(dve.tensor_tensor(eq_t[:], eq_t[:], wt_t[:], Alu.mult))
            step(dve.tensor_reduce(idf[:], eq_t[:], Ax.X, Alu.add))
            step(
                dve.tensor_copy(
                    idf8[:].rearrange("p (h u) -> p h u", h=8),
                    idf[:].unsqueeze(1).broadcast_to([128, 8, 16]),
                )
            )
            dve.wait_ge(s_mm, 1)
            step(dve.tensor_copy(idx32[:, :], t8[:]))
            step(
                dve.tensor_copy(
                    idx16[:, :],
                    idx32[:, :]
                    .bitcast(i16)
                    .rearrange("q (c two) -> q c two", two=2)[:, :, 0],
                )
            )

        @block.tensor
        def _(pe):
            pe.wait_ge(s_id, 16)
            pe.wait_ge(s_v, S_IDF)
            pe.transpose(t8[:], idf8[:], it_t[:]).then_inc(s_mm, 1)

        @block.gpsimd
        def _(pool):
            pool.load_library(mlp)
            pool.wait_ge(s_v, S_IDX)
            for k in range(NG - 1):
                q = k % 4
                cs = slice(k * cg, (k + 1) * cg)
                js = slice(k * jg, (k + 1) * jg)
                pool.dma_gather(
                    g_t[:, js, :], x_rows, idx16[:, cs], rows_g, rows_g, RW,
                    single_packet=True,
                    queue_num=q,
                ).then_inc(s_g[k], 16)
            # split the final chunk so the tail store is half-size and releases earlier
            for h in range(2):
                cs = slice(112 + 8 * h, 120 + 8 * h)
                js = slice(14 + h, 15 + h)
                pool.dma_gather(
                    g_t[:, js, :], x_rows, idx16[:, cs], 128, 128, RW,
                    single_packet=True,
                    queue_num=3,
                ).then_inc(s_g[7 + h], 16)

        @block.sync
        def _(sp):
            sp.wait_ge(s_v, 1)
            sp.dma_start(wt_t[:], wt.ap()).then_inc(s_wt, 16)
            sp.dma_start(it_t[:], it.ap()).then_inc(s_id, 16)
            for k in (0, 2, 4, 6):
                js = slice(k * jg, (k + 1) * jg)
                sp.wait_ge(s_g[k], 16)
                sp.dma_start(y_pjw[:, js, :], g_t[:, js, :]).then_inc(s_y, 16)
            sp.wait_ge(s_g[7], 16)
            sp.dma_start(y_pjw[:, 14:15, :], g_t[:, 14:15, :]).then_inc(s_y, 16)
            sp.wait_ge(s_y, 144)

    nc.compile()
    return nc


def _get_program():
    if "v2" not in _prog_cache:
        _prog_cache["v2"] = _build_program()
    return _prog_cache["v2"]


def _weights():
    p = np.arange(128, dtype=np.float32)[:, None, None]
    u = np.arange(16, dtype=np.float32)[None, :, None]
    r = np.arange(ROUTES, dtype=np.float32)[None, None, :]
    return np.ascontiguousarray(r + 8.0 * (p * 16.0 + u), dtype=np.float32)


def _identity():
    return np.eye(128, dtype=np.float32)


def _in_maps(inputs, routing_inputs):
    xb = inputs.astype(ml_dtypes.bfloat16)
    rt = np.ascontiguousarray(routing_inputs, dtype=np.float32)
    wt = _weights()
    it = _identity()
    return [
        {
            "x": np.ascontiguousarray(xb[c * B_SH : (c + 1) * B_SH]),
            "rt": np.ascontiguousarray(rt[c * B_SH : (c + 1) * B_SH]),
            "wt": wt,
            "it": it,
        }
        for c in range(N_CORES)
    ]


def _collect(res):
    return np.concatenate(
        [
            res.results[c]["y"].transpose(1, 0, 2).reshape(B_SH, RW)
            for c in range(N_CORES)
        ],
        axis=0,
    ).astype(np.float32)


def kernel(inputs: np.ndarray, routing_inputs: np.ndarray) -> np.ndarray:
    from concourse.bass_utils import run_bass_kernel_spmd

    nc = _get_program()
    in_maps = _in_maps(np.asarray(inputs), np.asarray(routing_inputs))
    res = None
    for attempt in range(3):
        try:
            res = run_bass_kernel_spmd(nc, in_maps, core_ids=list(range(N_CORES)))
            break
        except Exception:  # transient NRT_EXEC_UNIT_UNRECOVERABLE flakes
            if attempt == 2:
                raise
            import time

            time.sleep(2.0)
    return _collect(res)
